# revision 7
# baseline (speedup 1.0000x reference)
"""Trainium2 Bass kernel for DifferentiableSparseHypergraph (topk_masking).

Full computation per batch n:
  x_mean = x[n].mean(T)                      (C, V)
  q = Wq @ x_mean + bq                       (O=32, V)   [1x1 conv == matmul]
  q = q / max(||q||_2 over O, eps)
  H_raw = (q^T @ key_prototypes) / sqrt(O)   (V, M=128)
  topk10 -> softmax over the 10 vals -> scatter back; zeros elsewhere.

Kernel strategy (pure data-parallel over batch, 8 cores x 8 batches):
  * mean-over-T and the 1x1 conv are fused into PSUM-accumulated matmuls:
    psum[o, tl*64+v] += sum_c WqT[c,o] * x[c, t=8g+tl, v], accumulated over
    the 2 c-halves and 8 t-groups g => a final 8-way free-dim reduce gives
    sum_t (Wq @ x[:, t, :]).
  * L2 norm over channels is computed with a ones-matmul (partition-dim
    reduction on the PE), rsqrt on ACT+DVE.
  * top-10 per row is index-free: t_k = 10th largest per row (via the DVE
    max/match_replace/max top-8 primitives), and the output is
    exp(H) * (H >= t_k) / sum(exp(H) * (H >= t_k))  -- identical to
    softmax-over-topk scattered back (softmax is shift/subset invariant).
"""

import numpy as np

import concourse.bacc as bacc
import concourse.bass as bass
import concourse.mybir as mybir
import concourse.tile as tile

N, C, T, V = 64, 256, 64, 64
INTER = 32          # conv out channels
M = 128             # num hyperedges
TOPK = 10
NCORES = 8
FP = mybir.dt.float32
NEG_BIG = -1.0e30


def build_nc(nloc: int) -> bass.Bass:
    """Build the per-core Bass program processing `nloc` batches."""
    assert nloc % 2 == 0
    npair = nloc // 2
    # Bacc (not bare Bass): its compile()/finalize() pipeline splits
    # multi-semaphore waits into InstEventSemaphore pairs — walrus allows
    # at most one sync wait per regular instruction.
    nc = bacc.Bacc(target_bir_lowering=False, debug=False)

    x = nc.dram_tensor("x", (nloc, C, T, V), FP, kind="ExternalInput")
    wqt = nc.dram_tensor("wqt", (C, INTER), FP, kind="ExternalInput")
    kp = nc.dram_tensor("kp", (INTER, M), FP, kind="ExternalInput")
    bq = nc.dram_tensor("bq", (INTER, 1), FP, kind="ExternalInput")
    out = nc.dram_tensor("out", (nloc, V, M), FP, kind="ExternalOutput")

    A = mybir.AluOpType
    AF = mybir.ActivationFunctionType
    from concourse.tile import add_dep_helper

    with tile.TileContext(nc) as tc:
        with (
            tc.tile_pool(name="consts", bufs=1) as consts,
            tc.tile_pool(name="xp", bufs=2) as xp,
            tc.tile_pool(name="small", bufs=2) as small,
            tc.tile_pool(name="psA", bufs=2, space="PSUM") as psA,
            tc.tile_pool(name="psB", bufs=2, space="PSUM") as psB,
            tc.tile_pool(name="psS", bufs=1, space="PSUM") as psS,
        ):
            # --- replicated constants ---
            wq_sb = consts.tile([128, 2, INTER], FP)    # [c, c_half, o]
            nc.sync.dma_start(
                out=wq_sb[:], in_=wqt.rearrange("(h c) o -> c h o", h=2)
            )
            kp_sb = consts.tile([INTER, M], FP)
            nc.sync.dma_start(out=kp_sb[:], in_=kp[:])
            bq_sb = consts.tile([INTER, 1], FP)
            nc.sync.dma_start(out=bq_sb[:], in_=bq[:])
            ones_sb = consts.tile([INTER, 1], FP)
            nc.vector.memset(ones_sb[:], 1.0)

            # The fp32 self-loading matmul can carry at most ONE semaphore
            # wait (walrus S3_LW_STRUCT limit). Absorb the wq/kp DMA waits
            # with dummy 1x1 matmuls so the first real matmuls only wait on
            # their x-tile DMA.
            scr = psS.tile([1, 1], FP)
            d1 = nc.tensor.matmul(
                scr[:], wq_sb[:, 0, 0:1], wq_sb[:, 0, 0:1], start=True, stop=True
            )
            d2 = nc.tensor.matmul(
                scr[:], kp_sb[:, 0:1], kp_sb[:, 0:1], start=True, stop=True
            )
            add_dep_helper(d2.ins, d1.ins, sync=False, reason="pe-wait-absorb order")
            first_mm = None

            for p in range(npair):
                # one 8 MiB DMA per batch-pair: [c, b, h, (t v)]
                xt = xp.tile([128, 2, 2, T * V], FP, tag="xt")
                nc.sync.dma_start(
                    out=xt[:],
                    in_=x[2 * p : 2 * p + 2].rearrange(
                        "b (h c) t v -> c b h (t v)", h=2
                    ),
                )

                # fused mean-over-T + conv: accumulate over c-halves and
                # t-groups; psum free = (tl, v) partial t-sums
                q2 = small.tile([INTER, 2 * V], FP, tag="q2")
                for l in range(2):
                    pa = psA.tile([INTER, 512], FP, tag="pa")
                    for h in range(2):
                        for g in range(8):
                            mm = nc.tensor.matmul(
                                pa[:],
                                wq_sb[:, h, :],
                                xt[:, l, h, g * 512 : (g + 1) * 512],
                                start=(h == 0 and g == 0),
                                stop=(h == 1 and g == 7),
                            )
                            if first_mm is None:
                                first_mm = mm
                                add_dep_helper(
                                    mm.ins, d2.ins, sync=False,
                                    reason="pe-wait-absorb order",
                                )
                    qtmp = small.tile([INTER, V], FP, tag="qtmp")
                    nc.vector.reduce_sum(
                        out=qtmp[:],
                        in_=pa[:].rearrange("o (t v) -> o v t", t=8),
                        axis=mybir.AxisListType.X,
                    )
                    # q = qsum/T + bq
                    nc.vector.tensor_scalar(
                        out=q2[:, l * V : (l + 1) * V],
                        in0=qtmp[:],
                        scalar1=1.0 / T,
                        scalar2=bq_sb[:],
                        op0=A.mult,
                        op1=A.add,
                    )

                # scores: H[vv, m] = q2 normalized . kp * INTER^-0.5
                qsq = small.tile([INTER, 2 * V], FP, tag="qsq")
                nc.vector.tensor_mul(qsq[:], q2[:], q2[:])
                pb = psB.tile([2 * V, M], FP, tag="pb")
                nc.tensor.matmul(pb[:], q2[:], kp_sb[:], start=True, stop=True)
                pc = psB.tile([2 * V, 1], FP, tag="pc")
                nc.tensor.matmul(pc[:], qsq[:], ones_sb[:], start=True, stop=True)
                # rn = 1/sqrt(INTER * nsq) = INTER^-0.5 / ||q||
                nrm = small.tile([2 * V, 1], FP, tag="nrm")
                nc.scalar.activation(nrm[:], pc[:], AF.Sqrt, scale=float(INTER))
                rn = small.tile([2 * V, 1], FP, tag="rn")
                nc.vector.reciprocal(rn[:], nrm[:])
                H = small.tile([2 * V, M], FP, tag="H")
                nc.vector.tensor_scalar_mul(H[:], pb[:], rn[:])

                # t_k = 10th largest per row: top8, knock them out, top8 again
                top8a = small.tile([2 * V, 8], FP, tag="t8a")
                nc.vector.max(top8a[:], H[:])
                work = small.tile([2 * V, M], FP, tag="work")
                nc.vector.match_replace(work[:], top8a[:], H[:], NEG_BIG)
                top8b = small.tile([2 * V, 8], FP, tag="t8b")
                nc.vector.max(top8b[:], work[:])

                # masked softmax without scatter:
                # me = (H >= t_k) * exp(H); out = me / sum(me)
                e = small.tile([2 * V, M], FP, tag="e")
                nc.scalar.activation(e[:], H[:], AF.Exp)
                me = small.tile([2 * V, M], FP, tag="me")
                s = small.tile([2 * V, 1], FP, tag="s")
                nc.vector.scalar_tensor_tensor(
                    out=me[:],
                    in0=H[:],
                    scalar=top8b[:, 1:2],
                    in1=e[:],
                    op0=A.is_ge,
                    op1=A.mult,
                    accum_out=s[:],
                )
                r = small.tile([2 * V, 1], FP, tag="r")
                nc.vector.reciprocal(r[:], s[:])
                ot = small.tile([2 * V, M], FP, tag="ot")
                nc.vector.tensor_scalar_mul(ot[:], me[:], r[:])

                nc.sync.dma_start(
                    out=out[2 * p : 2 * p + 2].rearrange("b v m -> (b v) m"),
                    in_=ot[:],
                )
    nc.finalize()
    return nc


_NC_CACHE: dict[int, bass.Bass] = {}


def _get_nc(nloc: int) -> bass.Bass:
    if nloc not in _NC_CACHE:
        _NC_CACHE[nloc] = build_nc(nloc)
    return _NC_CACHE[nloc]


def _make_in_maps(x, Wq, bq, key_prototypes, ncores):
    nloc = x.shape[0] // ncores
    wqt = np.ascontiguousarray(np.asarray(Wq, dtype=np.float32).T)
    kpc = np.ascontiguousarray(np.asarray(key_prototypes, dtype=np.float32))
    bqc = np.ascontiguousarray(
        np.asarray(bq, dtype=np.float32).reshape(INTER, 1)
    )
    xc = np.asarray(x, dtype=np.float32)
    return [
        {
            "x": np.ascontiguousarray(xc[i * nloc : (i + 1) * nloc]),
            "wqt": wqt,
            "kp": kpc,
            "bq": bqc,
        }
        for i in range(ncores)
    ]


def run(inputs, trace: bool = False):
    """Run on hardware; returns (full_output, BassKernelResults)."""
    from concourse.bass_utils import run_bass_kernel_spmd

    x = inputs["x"]
    nloc = x.shape[0] // NCORES
    nc = _get_nc(nloc)
    in_maps = _make_in_maps(
        x, inputs["Wq"], inputs["bq"], inputs["key_prototypes"], NCORES
    )
    res = run_bass_kernel_spmd(nc, in_maps, list(range(NCORES)), trace=trace)
    out = np.concatenate([r["out"] for r in res.results], axis=0)
    return out, res


def kernel(**inputs) -> np.ndarray:
    out, _ = run(inputs, trace=False)
    return out


# revision 8
# speedup vs baseline: 1.1842x; 1.1842x over previous
"""Trainium2 Bass kernel for DifferentiableSparseHypergraph (topk_masking).

Full computation per batch n:
  x_mean = x[n].mean(T)                      (C, V)
  q = Wq @ x_mean + bq                       (O=32, V)   [1x1 conv == matmul]
  q = q / max(||q||_2 over O, eps)
  H_raw = (q^T @ key_prototypes) / sqrt(O)   (V, M=128)
  topk10 -> softmax over the 10 vals -> scatter back; zeros elsewhere.

Kernel strategy (pure data-parallel over batch, 8 cores x 8 batches):
  * mean-over-T and the 1x1 conv are fused into PSUM-accumulated matmuls:
    psum[o, tl*64+v] += sum_c WqT[c,o] * x[c, t=8g+tl, v], accumulated over
    the 2 c-halves and 8 t-groups g => a final 8-way free-dim reduce gives
    sum_t (Wq @ x[:, t, :]).
  * L2 norm over channels is computed with a ones-matmul (partition-dim
    reduction on the PE), rsqrt on ACT+DVE.
  * top-10 per row is index-free: t_k = 10th largest per row (via the DVE
    max/match_replace/max top-8 primitives), and the output is
    exp(H) * (H >= t_k) / sum(exp(H) * (H >= t_k))  -- identical to
    softmax-over-topk scattered back (softmax is shift/subset invariant).
"""

import numpy as np

import concourse.bacc as bacc
import concourse.bass as bass
import concourse.mybir as mybir
import concourse.tile as tile

N, C, T, V = 64, 256, 64, 64
INTER = 32          # conv out channels
M = 128             # num hyperedges
TOPK = 10
NCORES = 8
FP = mybir.dt.float32
NEG_BIG = -1.0e30


def build_nc(nloc: int) -> bass.Bass:
    """Build the per-core Bass program processing `nloc` batches."""
    assert nloc % 2 == 0
    npair = nloc // 2
    # Bacc (not bare Bass): its compile()/finalize() pipeline splits
    # multi-semaphore waits into InstEventSemaphore pairs — walrus allows
    # at most one sync wait per regular instruction.
    nc = bacc.Bacc(target_bir_lowering=False, debug=False)

    x = nc.dram_tensor("x", (nloc, C, T, V), FP, kind="ExternalInput")
    wqt = nc.dram_tensor("wqt", (C, INTER), FP, kind="ExternalInput")
    kp = nc.dram_tensor("kp", (INTER, M), FP, kind="ExternalInput")
    bq = nc.dram_tensor("bq", (INTER, 1), FP, kind="ExternalInput")
    out = nc.dram_tensor("out", (nloc, V, M), FP, kind="ExternalOutput")

    A = mybir.AluOpType
    AF = mybir.ActivationFunctionType
    from concourse.tile import add_dep_helper

    with tile.TileContext(nc) as tc:
        with (
            tc.tile_pool(name="consts", bufs=1) as consts,
            tc.tile_pool(name="xp", bufs=2) as xp,
            tc.tile_pool(name="small", bufs=2) as small,
            tc.tile_pool(name="psA", bufs=2, space="PSUM") as psA,
            tc.tile_pool(name="psB", bufs=2, space="PSUM") as psB,
            tc.tile_pool(name="psS", bufs=1, space="PSUM") as psS,
        ):
            # --- replicated constants ---
            wq_sb = consts.tile([128, 2, INTER], FP)    # [c, c_half, o]
            nc.sync.dma_start(
                out=wq_sb[:], in_=wqt.rearrange("(h c) o -> c h o", h=2)
            )
            kp_sb = consts.tile([INTER, M], FP)
            nc.sync.dma_start(out=kp_sb[:], in_=kp[:])
            bq_sb = consts.tile([INTER, 1], FP)
            nc.sync.dma_start(out=bq_sb[:], in_=bq[:])
            ones_sb = consts.tile([INTER, 1], FP)
            nc.vector.memset(ones_sb[:], 1.0)

            # The fp32 self-loading matmul can carry at most ONE semaphore
            # wait (walrus S3_LW_STRUCT limit). Absorb the wq/kp DMA waits
            # with dummy 1x1 matmuls so the first real matmuls only wait on
            # their x-tile DMA.
            scr = psS.tile([1, 1], FP)
            d1 = nc.tensor.matmul(
                scr[:], wq_sb[:, 0, 0:1], wq_sb[:, 0, 0:1], start=True, stop=True
            )
            d2 = nc.tensor.matmul(
                scr[:], kp_sb[:, 0:1], kp_sb[:, 0:1], start=True, stop=True
            )
            add_dep_helper(d2.ins, d1.ins, sync=False, reason="pe-wait-absorb order")
            first_mm = None

            q2 = None
            for n in range(nloc):
                # one 4 MiB DMA per batch: [c, h, (t v)]
                xt = xp.tile([128, 2, T * V], FP, tag="xt")
                nc.sync.dma_start(
                    out=xt[:],
                    in_=x[n].rearrange("(h c) t v -> c h (t v)", h=2),
                )

                # t-axis tree reduction on DVE (fp32 matmul passes are the
                # scarce resource: PE runs cold here): t 64 -> 32 -> 16
                xr1 = xp.tile([128, 2, T * V // 2], FP, tag="xr1")
                nc.vector.tensor_add(
                    xr1[:], xt[:, :, : T * V // 2], xt[:, :, T * V // 2 :]
                )
                xr2 = xp.tile([128, 2, T * V // 4], FP, tag="xr2")
                nc.vector.tensor_add(
                    xr2[:], xr1[:, :, : T * V // 4], xr1[:, :, T * V // 4 :]
                )

                # fused rest-of-mean + conv: accumulate over c-halves and
                # t-groups; psum free = (tl, v) partial t-sums
                l = n % 2
                if l == 0:
                    q2 = small.tile([INTER, 2 * V], FP, tag="q2")
                pa = psA.tile([INTER, 512], FP, tag="pa")
                for h in range(2):
                    for g in range(2):
                        mm = nc.tensor.matmul(
                            pa[:],
                            wq_sb[:, h, :],
                            xr2[:, h, g * 512 : (g + 1) * 512],
                            start=(h == 0 and g == 0),
                            stop=(h == 1 and g == 1),
                        )
                        if first_mm is None:
                            first_mm = mm
                            add_dep_helper(
                                mm.ins, d2.ins, sync=False,
                                reason="pe-wait-absorb order",
                            )
                qtmp = small.tile([INTER, V], FP, tag="qtmp")
                nc.vector.reduce_sum(
                    out=qtmp[:],
                    in_=pa[:].rearrange("o (t v) -> o v t", t=8),
                    axis=mybir.AxisListType.X,
                )
                # q = qsum/T + bq
                nc.vector.tensor_scalar(
                    out=q2[:, l * V : (l + 1) * V],
                    in0=qtmp[:],
                    scalar1=1.0 / T,
                    scalar2=bq_sb[:],
                    op0=A.mult,
                    op1=A.add,
                )
                if l == 0:
                    continue
                p = n // 2

                # scores: H[vv, m] = q2 normalized . kp * INTER^-0.5
                qsq = small.tile([INTER, 2 * V], FP, tag="qsq")
                nc.vector.tensor_mul(qsq[:], q2[:], q2[:])
                pb = psB.tile([2 * V, M], FP, tag="pb")
                nc.tensor.matmul(pb[:], q2[:], kp_sb[:], start=True, stop=True)
                pc = psB.tile([2 * V, 1], FP, tag="pc")
                nc.tensor.matmul(pc[:], qsq[:], ones_sb[:], start=True, stop=True)
                # rn = 1/sqrt(INTER * nsq) = INTER^-0.5 / ||q||
                nrm = small.tile([2 * V, 1], FP, tag="nrm")
                nc.scalar.activation(nrm[:], pc[:], AF.Sqrt, scale=float(INTER))
                rn = small.tile([2 * V, 1], FP, tag="rn")
                nc.vector.reciprocal(rn[:], nrm[:])
                H = small.tile([2 * V, M], FP, tag="H")
                nc.vector.tensor_scalar_mul(H[:], pb[:], rn[:])

                # t_k = 10th largest per row: top8, knock them out, top8 again
                top8a = small.tile([2 * V, 8], FP, tag="t8a")
                nc.vector.max(top8a[:], H[:])
                work = small.tile([2 * V, M], FP, tag="work")
                nc.vector.match_replace(work[:], top8a[:], H[:], NEG_BIG)
                top8b = small.tile([2 * V, 8], FP, tag="t8b")
                nc.vector.max(top8b[:], work[:])

                # masked softmax without scatter:
                # me = (H >= t_k) * exp(H); out = me / sum(me)
                e = small.tile([2 * V, M], FP, tag="e")
                nc.scalar.activation(e[:], H[:], AF.Exp)
                me = small.tile([2 * V, M], FP, tag="me")
                s = small.tile([2 * V, 1], FP, tag="s")
                nc.vector.scalar_tensor_tensor(
                    out=me[:],
                    in0=H[:],
                    scalar=top8b[:, 1:2],
                    in1=e[:],
                    op0=A.is_ge,
                    op1=A.mult,
                    accum_out=s[:],
                )
                r = small.tile([2 * V, 1], FP, tag="r")
                nc.vector.reciprocal(r[:], s[:])
                ot = small.tile([2 * V, M], FP, tag="ot")
                nc.vector.tensor_scalar_mul(ot[:], me[:], r[:])

                nc.sync.dma_start(
                    out=out[2 * p : 2 * p + 2].rearrange("b v m -> (b v) m"),
                    in_=ot[:],
                )
    nc.finalize()
    return nc


_NC_CACHE: dict[int, bass.Bass] = {}


def _get_nc(nloc: int) -> bass.Bass:
    if nloc not in _NC_CACHE:
        _NC_CACHE[nloc] = build_nc(nloc)
    return _NC_CACHE[nloc]


def _make_in_maps(x, Wq, bq, key_prototypes, ncores):
    nloc = x.shape[0] // ncores
    wqt = np.ascontiguousarray(np.asarray(Wq, dtype=np.float32).T)
    kpc = np.ascontiguousarray(np.asarray(key_prototypes, dtype=np.float32))
    bqc = np.ascontiguousarray(
        np.asarray(bq, dtype=np.float32).reshape(INTER, 1)
    )
    xc = np.asarray(x, dtype=np.float32)
    return [
        {
            "x": np.ascontiguousarray(xc[i * nloc : (i + 1) * nloc]),
            "wqt": wqt,
            "kp": kpc,
            "bq": bqc,
        }
        for i in range(ncores)
    ]


def run(inputs, trace: bool = False):
    """Run on hardware; returns (full_output, BassKernelResults)."""
    from concourse.bass_utils import run_bass_kernel_spmd

    x = inputs["x"]
    nloc = x.shape[0] // NCORES
    nc = _get_nc(nloc)
    in_maps = _make_in_maps(
        x, inputs["Wq"], inputs["bq"], inputs["key_prototypes"], NCORES
    )
    res = run_bass_kernel_spmd(nc, in_maps, list(range(NCORES)), trace=trace)
    out = np.concatenate([r["out"] for r in res.results], axis=0)
    return out, res


def kernel(**inputs) -> np.ndarray:
    out, _ = run(inputs, trace=False)
    return out


# revision 11
# speedup vs baseline: 1.3857x; 1.1702x over previous
"""Trainium2 Bass kernel for DifferentiableSparseHypergraph (topk_masking).

Full computation per batch n:
  x_mean = x[n].mean(T)                      (C, V)
  q = Wq @ x_mean + bq                       (O=32, V)   [1x1 conv == matmul]
  q = q / max(||q||_2 over O, eps)
  H_raw = (q^T @ key_prototypes) / sqrt(O)   (V, M=128)
  topk10 -> softmax over the 10 vals -> scatter back; zeros elsewhere.

Kernel strategy (pure data-parallel over batch, 8 cores x 8 batches):
  * mean-over-T and the 1x1 conv are fused into PSUM-accumulated matmuls:
    psum[o, tl*64+v] += sum_c WqT[c,o] * x[c, t=8g+tl, v], accumulated over
    the 2 c-halves and 8 t-groups g => a final 8-way free-dim reduce gives
    sum_t (Wq @ x[:, t, :]).
  * L2 norm over channels is computed with a ones-matmul (partition-dim
    reduction on the PE), rsqrt on ACT+DVE.
  * top-10 per row is index-free: t_k = 10th largest per row (via the DVE
    max/match_replace/max top-8 primitives), and the output is
    exp(H) * (H >= t_k) / sum(exp(H) * (H >= t_k))  -- identical to
    softmax-over-topk scattered back (softmax is shift/subset invariant).
"""

import numpy as np

import concourse.bacc as bacc
import concourse.bass as bass
import concourse.mybir as mybir
import concourse.tile as tile

N, C, T, V = 64, 256, 64, 64
INTER = 32          # conv out channels
M = 128             # num hyperedges
TOPK = 10
NCORES = 8
FP = mybir.dt.float32
NEG_BIG = -1.0e30


def build_nc(nloc: int) -> bass.Bass:
    """Build the per-core Bass program processing `nloc` batches."""
    assert nloc % 2 == 0
    npair = nloc // 2
    # Bacc (not bare Bass): its compile()/finalize() pipeline splits
    # multi-semaphore waits into InstEventSemaphore pairs — walrus allows
    # at most one sync wait per regular instruction.
    nc = bacc.Bacc(target_bir_lowering=False, debug=False)

    x = nc.dram_tensor("x", (nloc, C, T, V), FP, kind="ExternalInput")
    wqt = nc.dram_tensor("wqt", (C, INTER), FP, kind="ExternalInput")
    kp = nc.dram_tensor("kp", (INTER, M), FP, kind="ExternalInput")
    bq = nc.dram_tensor("bq", (INTER, 1), FP, kind="ExternalInput")
    out = nc.dram_tensor("out", (nloc, V, M), FP, kind="ExternalOutput")

    A = mybir.AluOpType
    AF = mybir.ActivationFunctionType
    from concourse.tile import add_dep_helper

    with tile.TileContext(nc) as tc:
        with (
            tc.tile_pool(name="consts", bufs=1) as consts,
            tc.tile_pool(name="xp", bufs=2) as xp,
            tc.tile_pool(name="small", bufs=2) as small,
            tc.tile_pool(name="psA", bufs=2, space="PSUM") as psA,
            tc.tile_pool(name="psB", bufs=2, space="PSUM") as psB,
            tc.tile_pool(name="psS", bufs=1, space="PSUM") as psS,
        ):
            # --- replicated constants ---
            wq_sb = consts.tile([128, 2, INTER], FP)    # [c, c_half, o]
            nc.sync.dma_start(
                out=wq_sb[:], in_=wqt.rearrange("(h c) o -> c h o", h=2)
            )
            kp_sb = consts.tile([INTER, M], FP)
            nc.sync.dma_start(out=kp_sb[:], in_=kp[:])
            bq_sb = consts.tile([INTER, 1], FP)
            nc.sync.dma_start(out=bq_sb[:], in_=bq[:])
            ones_sb = consts.tile([INTER, 1], FP)
            nc.vector.memset(ones_sb[:], 1.0)

            # The fp32 self-loading matmul can carry at most ONE semaphore
            # wait (walrus S3_LW_STRUCT limit). Absorb the wq/kp DMA waits
            # with dummy 1x1 matmuls so the first real matmuls only wait on
            # their x-tile DMA.
            scr = psS.tile([1, 1], FP)
            d1 = nc.tensor.matmul(
                scr[:], wq_sb[:, 0, 0:1], wq_sb[:, 0, 0:1], start=True, stop=True
            )
            d2 = nc.tensor.matmul(
                scr[:], kp_sb[:, 0:1], kp_sb[:, 0:1], start=True, stop=True
            )
            add_dep_helper(d2.ins, d1.ins, sync=False, reason="pe-wait-absorb order")
            first_mm = None

            q2 = None
            for n in range(nloc):
                # per-c-half 2 MiB DMAs into separate tiles so each half's
                # reduction pipeline starts as soon as its bytes land
                xh = []
                for h in range(2):
                    t = xp.tile([128, T * V], FP, tag=f"xh{h}")
                    nc.sync.dma_start(
                        out=t[:], in_=x[n, h * 128 : (h + 1) * 128]
                    )
                    xh.append(t)

                # t-axis tree reduction on DVE (fp32 matmul passes are the
                # scarce resource: PE runs cold here): t 64 -> 32 -> 16 -> 8
                xr3 = []
                for h in range(2):
                    r1 = xp.tile([128, T * V // 2], FP, tag=f"r1{h}")
                    nc.vector.tensor_add(
                        r1[:], xh[h][:, : T * V // 2], xh[h][:, T * V // 2 :]
                    )
                    r2 = xp.tile([128, T * V // 4], FP, tag=f"r2{h}")
                    nc.vector.tensor_add(
                        r2[:], r1[:, : T * V // 4], r1[:, T * V // 4 :]
                    )
                    r3 = xp.tile([128, T * V // 8], FP, tag=f"r3{h}")
                    nc.vector.tensor_add(
                        r3[:], r2[:, : T * V // 8], r2[:, T * V // 8 :]
                    )
                    xr3.append(r3)

                # fused rest-of-mean + conv: accumulate over c-halves;
                # psum free = (tl, v) partial t-sums
                l = n % 2
                if l == 0:
                    q2 = small.tile([INTER, 2 * V], FP, tag="q2")
                pa = psA.tile([INTER, 512], FP, tag="pa")
                for h in range(2):
                    mm = nc.tensor.matmul(
                        pa[:],
                        wq_sb[:, h, :],
                        xr3[h][:],
                        start=(h == 0),
                        stop=(h == 1),
                    )
                    if first_mm is None:
                        first_mm = mm
                        add_dep_helper(
                            mm.ins, d2.ins, sync=False,
                            reason="pe-wait-absorb order",
                        )
                qtmp = small.tile([INTER, V], FP, tag="qtmp")
                nc.vector.reduce_sum(
                    out=qtmp[:],
                    in_=pa[:].rearrange("o (t v) -> o v t", t=8),
                    axis=mybir.AxisListType.X,
                )
                # q = qsum/T + bq
                nc.vector.tensor_scalar(
                    out=q2[:, l * V : (l + 1) * V],
                    in0=qtmp[:],
                    scalar1=1.0 / T,
                    scalar2=bq_sb[:],
                    op0=A.mult,
                    op1=A.add,
                )
                if l == 0:
                    continue
                p = n // 2

                # scores: H[vv, m] = q2 normalized . kp * INTER^-0.5
                qsq = small.tile([INTER, 2 * V], FP, tag="qsq")
                nc.vector.tensor_mul(qsq[:], q2[:], q2[:])
                pb = psB.tile([2 * V, M], FP, tag="pb")
                nc.tensor.matmul(pb[:], q2[:], kp_sb[:], start=True, stop=True)
                pc = psB.tile([2 * V, 1], FP, tag="pc")
                nc.tensor.matmul(pc[:], qsq[:], ones_sb[:], start=True, stop=True)
                # rn = 1/sqrt(INTER * nsq) = INTER^-0.5 / ||q||
                nrm = small.tile([2 * V, 1], FP, tag="nrm")
                nc.scalar.activation(nrm[:], pc[:], AF.Sqrt, scale=float(INTER))
                rn = small.tile([2 * V, 1], FP, tag="rn")
                nc.vector.reciprocal(rn[:], nrm[:])
                H = small.tile([2 * V, M], FP, tag="H")
                nc.vector.tensor_scalar_mul(H[:], pb[:], rn[:])

                # t_k = 10th largest per row: top8, knock them out, top8 again
                top8a = small.tile([2 * V, 8], FP, tag="t8a")
                nc.vector.max(top8a[:], H[:])
                work = small.tile([2 * V, M], FP, tag="work")
                nc.vector.match_replace(work[:], top8a[:], H[:], NEG_BIG)
                top8b = small.tile([2 * V, 8], FP, tag="t8b")
                nc.vector.max(top8b[:], work[:])

                # masked softmax without scatter:
                # me = (H >= t_k) * exp(H); out = me / sum(me)
                e = small.tile([2 * V, M], FP, tag="e")
                nc.scalar.activation(e[:], H[:], AF.Exp)
                me = small.tile([2 * V, M], FP, tag="me")
                s = small.tile([2 * V, 1], FP, tag="s")
                nc.vector.scalar_tensor_tensor(
                    out=me[:],
                    in0=H[:],
                    scalar=top8b[:, 1:2],
                    in1=e[:],
                    op0=A.is_ge,
                    op1=A.mult,
                    accum_out=s[:],
                )
                r = small.tile([2 * V, 1], FP, tag="r")
                nc.vector.reciprocal(r[:], s[:])
                ot = small.tile([2 * V, M], FP, tag="ot")
                nc.vector.tensor_scalar_mul(ot[:], me[:], r[:])

                nc.sync.dma_start(
                    out=out[2 * p : 2 * p + 2].rearrange("b v m -> (b v) m"),
                    in_=ot[:],
                )
    nc.finalize()
    return nc


_NC_CACHE: dict[int, bass.Bass] = {}


def _get_nc(nloc: int) -> bass.Bass:
    if nloc not in _NC_CACHE:
        _NC_CACHE[nloc] = build_nc(nloc)
    return _NC_CACHE[nloc]


def _make_in_maps(x, Wq, bq, key_prototypes, ncores):
    nloc = x.shape[0] // ncores
    wqt = np.ascontiguousarray(np.asarray(Wq, dtype=np.float32).T)
    kpc = np.ascontiguousarray(np.asarray(key_prototypes, dtype=np.float32))
    bqc = np.ascontiguousarray(
        np.asarray(bq, dtype=np.float32).reshape(INTER, 1)
    )
    xc = np.asarray(x, dtype=np.float32)
    return [
        {
            "x": np.ascontiguousarray(xc[i * nloc : (i + 1) * nloc]),
            "wqt": wqt,
            "kp": kpc,
            "bq": bqc,
        }
        for i in range(ncores)
    ]


def run(inputs, trace: bool = False):
    """Run on hardware; returns (full_output, BassKernelResults)."""
    from concourse.bass_utils import run_bass_kernel_spmd

    x = inputs["x"]
    nloc = x.shape[0] // NCORES
    nc = _get_nc(nloc)
    in_maps = _make_in_maps(
        x, inputs["Wq"], inputs["bq"], inputs["key_prototypes"], NCORES
    )
    res = run_bass_kernel_spmd(nc, in_maps, list(range(NCORES)), trace=trace)
    out = np.concatenate([r["out"] for r in res.results], axis=0)
    return out, res


def kernel(**inputs) -> np.ndarray:
    out, _ = run(inputs, trace=False)
    return out
